# revision 2
# baseline (speedup 1.0000x reference)
"""Trainium2 Bass kernel for nn_PoolNU: gather + max-pool over neighbour table.

reference:
    x: (8, 128, 65536) f32, neighbours: (9, 16384) int
    out[b, c, j] = max_k x[b, c, neighbours[k, j]]

Strategy:
    - The neighbour table is shared across (b, c), so one gathered "row"
      carries ALL batches and channels for a location: x repacked host-side to
      (65536, B*C=1024). Values are cast to bf16 (tolerance is 2e-2; bf16
      rounding is ~0.2%), halving all HBM traffic: 2KB rows instead of 4KB.
    - Output locations (16384) are sharded across the 8 NeuronCores (2048
      per core). Each core needs at most 9*2048=18432 distinct source rows,
      which the host compacts into a per-core x_sub with remapped indices —
      guaranteed to fit dma_gather's int16 index window (< 32768).
    - Device per tile of 128 locations: one 1152-index dma_gather (all 9
      slots), vector max-tree over the 9 slots in bf16, store 2KB rows.
    - Host reassembles (core, loc, b, c) -> (b, c, loc) and casts to f32.
"""

import sys

sys.path.insert(0, "/opt/trn_rl_repo")

import ml_dtypes
import numpy as np

import concourse.mybir as mybir
from concourse import bacc, bass_utils
from concourse.tile import TileContext

B = 8
C = 128
LIN = 65536
K = 9
LOUT = 16384

P = 128
NCORE = 8
LPC = LOUT // NCORE          # locations per core (2048)
NTILE = LPC // P             # tiles per core (16)
E = B * C                    # elements per gathered row (1024)
UMAX = K * LPC               # padded x_sub rows (18432)
NIDX = K * P                 # indices per tile call (1152)
WT = NIDX // 16              # idx cols per tile (72)

BF16 = mybir.dt.bfloat16

_CACHE = {}


def _build_program():
    nc = bacc.Bacc("TRN2", target_bir_lowering=False, debug=False, num_devices=1)

    xs = nc.dram_tensor("xs", [UMAX, E], BF16, kind="ExternalInput")
    idx = nc.dram_tensor("idx", [P, NTILE * WT], mybir.dt.int16,
                         kind="ExternalInput")
    out = nc.dram_tensor("out", [LPC, E], BF16, kind="ExternalOutput")

    with TileContext(nc) as tc:
        with tc.tile_pool(name="sbuf", bufs=2) as pool:
            idx_sb = pool.tile([P, NTILE * WT], mybir.dt.int16, bufs=1)
            nc.sync.dma_start(out=idx_sb[:], in_=idx.ap())

            for t in range(NTILE):
                g = pool.tile([P, K * E], BF16, tag="g")
                nc.gpsimd.dma_gather(
                    out_ap=g[:].rearrange("p (g e) -> p g e", e=E),
                    in_ap=xs.ap(),
                    idxs_ap=idx_sb[:, t * WT : (t + 1) * WT],
                    num_idxs=NIDX,
                    num_idxs_reg=NIDX,
                    elem_size=E,
                )
                t4 = pool.tile([P, 4 * E], BF16, tag="t4")
                nc.vector.tensor_tensor(
                    out=t4[:], in0=g[:, : 4 * E], in1=g[:, 4 * E : 8 * E],
                    op=mybir.AluOpType.max,
                )
                t2 = pool.tile([P, 2 * E], BF16, tag="t2")
                nc.vector.tensor_tensor(
                    out=t2[:], in0=t4[:, : 2 * E], in1=t4[:, 2 * E :],
                    op=mybir.AluOpType.max,
                )
                acc = pool.tile([P, E], BF16, tag="acc")
                nc.vector.tensor_tensor(
                    out=acc[:], in0=t2[:, :E], in1=t2[:, E:],
                    op=mybir.AluOpType.max,
                )
                nc.vector.tensor_tensor(
                    out=acc[:], in0=acc[:], in1=g[:, 8 * E :],
                    op=mybir.AluOpType.max,
                )
                nc.sync.dma_start(
                    out=out.ap()[t * P : (t + 1) * P, :], in_=acc[:]
                )

    nc.compile()
    return nc


def _get_program():
    if "nc" not in _CACHE:
        _CACHE["nc"] = _build_program()
    return _CACHE["nc"]


def _wrap16(lst: np.ndarray) -> np.ndarray:
    """(N,) int -> (128, N/16) int16: 16-partition wrap, replicated x8."""
    w = len(lst) // 16
    return np.tile(lst.reshape(w, 16).T, (8, 1)).astype(np.int16)


def kernel(x: np.ndarray, neighbours: np.ndarray) -> np.ndarray:
    x = np.asarray(x)
    nb = np.asarray(neighbours).astype(np.int64)          # (K, LOUT)
    assert x.shape == (B, C, LIN) and x.dtype == np.float32
    assert nb.shape == (K, LOUT)

    # (LIN, B*C) bf16: one 2KB row per input location
    xm = np.ascontiguousarray(x.transpose(2, 0, 1).reshape(LIN, E)).astype(
        ml_dtypes.bfloat16
    )

    in_maps = []
    for core in range(NCORE):
        nbc = nb[:, core * LPC : (core + 1) * LPC]        # (K, LPC)
        uniq, inv = np.unique(nbc, return_inverse=True)
        inv = inv.reshape(K, LPC)
        xs = np.zeros((UMAX, E), dtype=ml_dtypes.bfloat16)
        xs[: len(uniq)] = xm[uniq]
        cols = []
        for t in range(NTILE):
            # tile call: entry s*128+p -> inv[s, t*128+p]
            cols.append(_wrap16(inv[:, t * P : (t + 1) * P].ravel()))
        idx_np = np.ascontiguousarray(np.concatenate(cols, axis=1))
        in_maps.append({"xs": xs, "idx": idx_np})

    nc = _get_program()
    res = bass_utils.run_bass_kernel_spmd(nc, in_maps, core_ids=list(range(NCORE)))
    _CACHE["last_result"] = res

    # out per core: (LPC, B*C) bf16 -> full (B, C, LOUT) f32
    dev = np.concatenate(
        [np.asarray(res.results[c]["out"]) for c in range(NCORE)]
    )  # (LOUT, E) bf16
    return np.ascontiguousarray(
        dev.reshape(LOUT, B, C).transpose(1, 2, 0)
    ).astype(np.float32)


# revision 4
# speedup vs baseline: 1.3484x; 1.3484x over previous
"""Trainium2 Bass kernel for nn_PoolNU: gather + max-pool over neighbour table.

reference:
    x: (8, 128, 65536) f32, neighbours: (9, 16384) int
    out[b, c, j] = max_k x[b, c, neighbours[k, j]]

Strategy:
    - The neighbour table is shared across (b, c), so one gathered "row"
      carries ALL batches and channels for a location: x repacked host-side to
      (65536, B*C=1024). Values are cast to bf16 (tolerance is 2e-2; bf16
      rounding is ~0.2%), halving all HBM traffic: 2KB rows instead of 4KB.
    - Output locations (16384) are sharded across the 8 NeuronCores (2048
      per core). Each core needs at most 9*2048=18432 distinct source rows,
      which the host compacts into a per-core x_sub with remapped indices —
      guaranteed to fit dma_gather's int16 index window (< 32768).
    - dma_gather is limited to 1024 indices per call (64-descriptor packet
      ceiling per SDMA lane with single_packet). Device per tile of 128
      locations: gather slots 0-7 (1024 idxs), slot 8 gathered per quarter
      (512 idxs), vector max-tree in bf16, store 2KB rows.
    - Host reassembles (core, loc, b, c) -> (b, c, loc) and casts to f32.
"""

import sys

sys.path.insert(0, "/opt/trn_rl_repo")

import ml_dtypes
import numpy as np

import concourse.mybir as mybir
from concourse import bacc, bass_utils
from concourse.tile import TileContext

B = 8
C = 128
LIN = 65536
K = 9
LOUT = 16384

P = 128
NCORE = 8
LPC = LOUT // NCORE          # locations per core (2048)
NTILE = LPC // P             # tiles per core (16)
E = B * C                    # elements per gathered row (1024)
UMAX = K * LPC               # padded x_sub rows (18432)
NMAX = 1024                  # max indices per dma_gather call

BF16 = mybir.dt.bfloat16

WA = NMAX // 16              # 64 idx cols per tile call
WQ = 4 * P // 16             # 32 idx cols per quarter slot-8 call
NQ = NTILE // 4

_CACHE = {}


def _build_program():
    nc = bacc.Bacc("TRN2", target_bir_lowering=False, debug=False, num_devices=1)

    xs = nc.dram_tensor("xs", [UMAX, E], BF16, kind="ExternalInput")
    # idx layout per core: per tile one 1024-index call (slots 0..7), then per
    # quarter (4 tiles) one 512-index call for slot 8. All 16-wrapped and
    # replicated over the 128 partitions in groups of 16.
    idx = nc.dram_tensor("idx", [P, NTILE * WA + NQ * WQ], mybir.dt.int16,
                         kind="ExternalInput")
    out = nc.dram_tensor("out", [LPC, E], BF16, kind="ExternalOutput")

    with TileContext(nc) as tc:
        with tc.tile_pool(name="sbuf", bufs=2) as pool:
            idx_sb = pool.tile([P, NTILE * WA + NQ * WQ], mybir.dt.int16, bufs=1)
            nc.sync.dma_start(out=idx_sb[:], in_=idx.ap())

            for q in range(NQ):
                s8 = pool.tile([P, 4 * E], BF16, tag="s8")
                cq = NTILE * WA + q * WQ
                nc.gpsimd.dma_gather(
                    out_ap=s8[:].rearrange("p (g e) -> p g e", e=E),
                    in_ap=xs.ap(),
                    idxs_ap=idx_sb[:, cq : cq + WQ],
                    num_idxs=4 * P,
                    num_idxs_reg=4 * P,
                    elem_size=E,
                )
                for ti in range(4):
                    t = q * 4 + ti
                    g = pool.tile([P, 8 * E], BF16, tag="g")
                    c0 = t * WA
                    nc.gpsimd.dma_gather(
                        out_ap=g[:].rearrange("p (g e) -> p g e", e=E),
                        in_ap=xs.ap(),
                        idxs_ap=idx_sb[:, c0 : c0 + WA],
                        num_idxs=NMAX,
                        num_idxs_reg=NMAX,
                        elem_size=E,
                    )
                    t4 = pool.tile([P, 4 * E], BF16, tag="t4")
                    nc.vector.tensor_tensor(
                        out=t4[:], in0=g[:, : 4 * E], in1=g[:, 4 * E :],
                        op=mybir.AluOpType.max,
                    )
                    t2 = pool.tile([P, 2 * E], BF16, tag="t2")
                    nc.vector.tensor_tensor(
                        out=t2[:], in0=t4[:, : 2 * E], in1=t4[:, 2 * E :],
                        op=mybir.AluOpType.max,
                    )
                    acc = pool.tile([P, E], BF16, tag="acc")
                    nc.vector.tensor_tensor(
                        out=acc[:], in0=t2[:, :E], in1=t2[:, E:],
                        op=mybir.AluOpType.max,
                    )
                    nc.vector.tensor_tensor(
                        out=acc[:], in0=acc[:], in1=s8[:, ti * E : (ti + 1) * E],
                        op=mybir.AluOpType.max,
                    )
                    nc.sync.dma_start(
                        out=out.ap()[t * P : (t + 1) * P, :], in_=acc[:]
                    )

    nc.compile()
    return nc


def _get_program():
    if "nc" not in _CACHE:
        _CACHE["nc"] = _build_program()
    return _CACHE["nc"]


def _wrap16(lst: np.ndarray) -> np.ndarray:
    """(N,) int -> (128, N/16) int16: 16-partition wrap, replicated x8."""
    w = len(lst) // 16
    return np.tile(lst.reshape(w, 16).T, (8, 1)).astype(np.int16)


def _host_prepare(x: np.ndarray, nb: np.ndarray) -> list[dict]:
    # (LIN, B*C) bf16: one 2KB row per input location
    xm = np.ascontiguousarray(x.transpose(2, 0, 1).reshape(LIN, E)).astype(
        ml_dtypes.bfloat16
    )

    in_maps = []
    for core in range(NCORE):
        nbc = nb[:, core * LPC : (core + 1) * LPC]        # (K, LPC)
        uniq, inv = np.unique(nbc, return_inverse=True)
        inv = inv.reshape(K, LPC)
        xs = np.zeros((UMAX, E), dtype=ml_dtypes.bfloat16)
        xs[: len(uniq)] = xm[uniq]
        cols = []
        for t in range(NTILE):
            loc2d = inv[:, t * P : (t + 1) * P]           # (K, P) local idx
            # per-tile call: slots 0..7 -> list[s*128+p] = loc2d[s, p]
            cols.append(_wrap16(loc2d[:8].ravel()))
        for q in range(NQ):
            # per-quarter slot-8 call: list[g*128+p] = inv[8, (q*4+g)*P + p]
            cols.append(_wrap16(inv[8, q * 4 * P : (q + 1) * 4 * P]))
        idx_np = np.ascontiguousarray(np.concatenate(cols, axis=1))
        in_maps.append({"xs": xs, "idx": idx_np})
    return in_maps


def kernel(x: np.ndarray, neighbours: np.ndarray) -> np.ndarray:
    x = np.asarray(x)
    nb = np.asarray(neighbours).astype(np.int64)          # (K, LOUT)
    assert x.shape == (B, C, LIN) and x.dtype == np.float32
    assert nb.shape == (K, LOUT)

    in_maps = _host_prepare(x, nb)
    nc = _get_program()
    res = bass_utils.run_bass_kernel_spmd(nc, in_maps, core_ids=list(range(NCORE)))
    _CACHE["last_result"] = res

    # out per core: (LPC, B*C) bf16 -> full (B, C, LOUT) f32
    dev = np.concatenate(
        [np.asarray(res.results[c]["out"]) for c in range(NCORE)]
    )  # (LOUT, E) bf16
    return np.ascontiguousarray(
        dev.reshape(LOUT, B, C).transpose(1, 2, 0)
    ).astype(np.float32)


# revision 6
# speedup vs baseline: 1.4722x; 1.0918x over previous
"""Trainium2 Bass kernel for nn_PoolNU: gather + max-pool over neighbour table.

reference:
    x: (8, 128, 65536) f32, neighbours: (9, 16384) int
    out[b, c, j] = max_k x[b, c, neighbours[k, j]]

Strategy:
    - The neighbour table is shared across (b, c), so one gathered "row"
      carries ALL batches and channels for a location: x repacked host-side to
      (65536, B*C=1024). Values are cast to bf16 (tolerance is 2e-2; bf16
      rounding is ~0.2%), halving all HBM traffic: 2KB rows instead of 4KB.
    - Output locations (16384) are sharded across the 8 NeuronCores (2048
      per core). Each core needs at most 9*2048=18432 distinct source rows,
      which the host compacts into a per-core x_sub with remapped indices —
      guaranteed to fit dma_gather's int16 index window (< 32768).
    - dma_gather is limited to 1024 indices per call (64-descriptor packet
      ceiling per SDMA lane with single_packet). Device per tile of 128
      locations: gather slots 0-7 (1024 idxs), slot 8 gathered per quarter
      (512 idxs), vector max-tree in bf16, store 2KB rows.
    - Host reassembles (core, loc, b, c) -> (b, c, loc) and casts to f32.
"""

import sys

sys.path.insert(0, "/opt/trn_rl_repo")

import ml_dtypes
import numpy as np

import concourse.mybir as mybir
from concourse import bacc, bass_utils
from concourse.tile import TileContext

B = 8
C = 128
LIN = 65536
K = 9
LOUT = 16384

P = 128
NCORE = 8
LPC = LOUT // NCORE          # locations per core (2048)
NTILE = LPC // P             # tiles per core (16)
E = B * C                    # elements per gathered row (1024)
UMAX = K * LPC               # padded x_sub rows (18432)
NMAX = 1024                  # max indices per dma_gather call

BF16 = mybir.dt.bfloat16

WA = NMAX // 16              # 64 idx cols per tile call
WQ = 4 * P // 16             # 32 idx cols per quarter slot-8 call
NQ = NTILE // 4

_CACHE = {}


NQUEUES = 4


def _build_program():
    nc = bacc.Bacc(
        "TRN2",
        target_bir_lowering=False,
        debug=False,
        num_devices=1,
        num_swdge_queues=NQUEUES,
    )

    xs = nc.dram_tensor("xs", [UMAX, E], BF16, kind="ExternalInput")
    # idx layout per core: per tile one 1024-index call (slots 0..7), then per
    # quarter (4 tiles) one 512-index call for slot 8. All 16-wrapped and
    # replicated over the 128 partitions in groups of 16.
    idx = nc.dram_tensor("idx", [P, NTILE * WA + NQ * WQ], mybir.dt.int16,
                         kind="ExternalInput")
    out = nc.dram_tensor("out", [LPC, E], BF16, kind="ExternalOutput")

    with TileContext(nc) as tc:
        with tc.tile_pool(name="sbuf", bufs=2) as pool:
            idx_sb = pool.tile([P, NTILE * WA + NQ * WQ], mybir.dt.int16, bufs=1)
            nc.sync.dma_start(out=idx_sb[:], in_=idx.ap())

            call_i = 0
            for q in range(NQ):
                s8 = pool.tile([P, 4 * E], BF16, tag="s8")
                cq = NTILE * WA + q * WQ
                nc.gpsimd.dma_gather(
                    out_ap=s8[:].rearrange("p (g e) -> p g e", e=E),
                    in_ap=xs.ap(),
                    idxs_ap=idx_sb[:, cq : cq + WQ],
                    num_idxs=4 * P,
                    num_idxs_reg=4 * P,
                    elem_size=E,
                    queue_num=call_i % NQUEUES,
                )
                call_i += 1
                for ti in range(4):
                    t = q * 4 + ti
                    g = pool.tile([P, 8 * E], BF16, tag="g")
                    c0 = t * WA
                    nc.gpsimd.dma_gather(
                        out_ap=g[:].rearrange("p (g e) -> p g e", e=E),
                        in_ap=xs.ap(),
                        idxs_ap=idx_sb[:, c0 : c0 + WA],
                        num_idxs=NMAX,
                        num_idxs_reg=NMAX,
                        elem_size=E,
                        queue_num=call_i % NQUEUES,
                    )
                    call_i += 1
                    t4 = pool.tile([P, 4 * E], BF16, tag="t4")
                    nc.vector.tensor_tensor(
                        out=t4[:], in0=g[:, : 4 * E], in1=g[:, 4 * E :],
                        op=mybir.AluOpType.max,
                    )
                    t2 = pool.tile([P, 2 * E], BF16, tag="t2")
                    nc.vector.tensor_tensor(
                        out=t2[:], in0=t4[:, : 2 * E], in1=t4[:, 2 * E :],
                        op=mybir.AluOpType.max,
                    )
                    acc = pool.tile([P, E], BF16, tag="acc")
                    nc.vector.tensor_tensor(
                        out=acc[:], in0=t2[:, :E], in1=t2[:, E:],
                        op=mybir.AluOpType.max,
                    )
                    nc.vector.tensor_tensor(
                        out=acc[:], in0=acc[:], in1=s8[:, ti * E : (ti + 1) * E],
                        op=mybir.AluOpType.max,
                    )
                    nc.sync.dma_start(
                        out=out.ap()[t * P : (t + 1) * P, :], in_=acc[:]
                    )

    nc.compile()
    return nc


def _get_program():
    if "nc" not in _CACHE:
        _CACHE["nc"] = _build_program()
    return _CACHE["nc"]


def _wrap16(lst: np.ndarray) -> np.ndarray:
    """(N,) int -> (128, N/16) int16: 16-partition wrap, replicated x8."""
    w = len(lst) // 16
    return np.tile(lst.reshape(w, 16).T, (8, 1)).astype(np.int16)


def _host_prepare(x: np.ndarray, nb: np.ndarray) -> list[dict]:
    # (LIN, B*C) bf16: one 2KB row per input location
    xm = np.ascontiguousarray(x.transpose(2, 0, 1).reshape(LIN, E)).astype(
        ml_dtypes.bfloat16
    )

    in_maps = []
    for core in range(NCORE):
        nbc = nb[:, core * LPC : (core + 1) * LPC]        # (K, LPC)
        uniq, inv = np.unique(nbc, return_inverse=True)
        inv = inv.reshape(K, LPC)
        xs = np.zeros((UMAX, E), dtype=ml_dtypes.bfloat16)
        xs[: len(uniq)] = xm[uniq]
        cols = []
        for t in range(NTILE):
            loc2d = inv[:, t * P : (t + 1) * P]           # (K, P) local idx
            # per-tile call: slots 0..7 -> list[s*128+p] = loc2d[s, p]
            cols.append(_wrap16(loc2d[:8].ravel()))
        for q in range(NQ):
            # per-quarter slot-8 call: list[g*128+p] = inv[8, (q*4+g)*P + p]
            cols.append(_wrap16(inv[8, q * 4 * P : (q + 1) * 4 * P]))
        idx_np = np.ascontiguousarray(np.concatenate(cols, axis=1))
        in_maps.append({"xs": xs, "idx": idx_np})
    return in_maps


def kernel(x: np.ndarray, neighbours: np.ndarray) -> np.ndarray:
    x = np.asarray(x)
    nb = np.asarray(neighbours).astype(np.int64)          # (K, LOUT)
    assert x.shape == (B, C, LIN) and x.dtype == np.float32
    assert nb.shape == (K, LOUT)

    in_maps = _host_prepare(x, nb)
    nc = _get_program()
    res = bass_utils.run_bass_kernel_spmd(nc, in_maps, core_ids=list(range(NCORE)))
    _CACHE["last_result"] = res

    # out per core: (LPC, B*C) bf16 -> full (B, C, LOUT) f32
    dev = np.concatenate(
        [np.asarray(res.results[c]["out"]) for c in range(NCORE)]
    )  # (LOUT, E) bf16
    return np.ascontiguousarray(
        dev.reshape(LOUT, B, C).transpose(1, 2, 0)
    ).astype(np.float32)


# revision 9
# speedup vs baseline: 1.7212x; 1.1691x over previous
"""Trainium2 Bass kernel for nn_PoolNU: gather + max-pool over neighbour table.

reference:
    x: (8, 128, 65536) f32, neighbours: (9, 16384) int
    out[b, c, j] = max_k x[b, c, neighbours[k, j]]

Strategy:
    - The neighbour table is shared across (b, c), so one gathered "row"
      carries ALL batches and channels for a location: x repacked host-side to
      (65536, B*C=1024). Values are cast to bf16 (tolerance is 2e-2; bf16
      rounding is ~0.2%), halving all HBM traffic: 2KB rows instead of 4KB.
    - Output locations (16384) are sharded across the 8 NeuronCores (2048
      per core). Each core needs at most 9*2048=18432 distinct source rows,
      which the host compacts into a per-core x_sub with remapped indices —
      guaranteed to fit dma_gather's int16 index window (< 32768).
    - dma_gather is limited to 1024 indices per call (64-descriptor packet
      ceiling per SDMA lane with single_packet). Device per tile of 128
      locations: gather slots 0-7 (1024 idxs), slot 8 gathered per quarter
      (512 idxs), vector max-tree in bf16, store 2KB rows.
    - Host reassembles (core, loc, b, c) -> (b, c, loc) and casts to f32.
"""

import sys

sys.path.insert(0, "/opt/trn_rl_repo")

import ml_dtypes
import numpy as np

import concourse.mybir as mybir
from concourse import bacc, bass_utils
from concourse.tile import TileContext

B = 8
C = 128
LIN = 65536
K = 9
LOUT = 16384

P = 128
NCORE = 8
LPC = LOUT // NCORE          # locations per core (2048)
NTILE = LPC // P             # tiles per core (16)
E = B * C                    # elements per gathered row (1024)
UMAX = K * LPC               # padded x_sub rows (18432)
NMAX = 1024                  # max indices per dma_gather call

BF16 = mybir.dt.bfloat16

WA = NMAX // 16              # 64 idx cols per tile call
WQ = 8 * P // 16             # 64 idx cols per half slot-8 call (1024 idxs)
NQ = NTILE // 8              # two slot-8 calls, each covering 8 tiles

_CACHE = {}


NQUEUES = 4


def _build_program():
    nc = bacc.Bacc(
        "TRN2",
        target_bir_lowering=False,
        debug=False,
        num_devices=1,
        num_swdge_queues=NQUEUES,
    )

    xs = nc.dram_tensor("xs", [UMAX, E], BF16, kind="ExternalInput")
    # idx layout per core: per tile one 1024-index call (slots 0..7), then per
    # quarter (4 tiles) one 512-index call for slot 8. All 16-wrapped and
    # replicated over the 128 partitions in groups of 16.
    idx = nc.dram_tensor("idx", [P, NTILE * WA + NQ * WQ], mybir.dt.int16,
                         kind="ExternalInput")
    out = nc.dram_tensor("out", [LPC, E], BF16, kind="ExternalOutput")

    with TileContext(nc) as tc:
        with tc.tile_pool(name="sbuf", bufs=2) as pool:
            idx_sb = pool.tile([P, NTILE * WA + NQ * WQ], mybir.dt.int16, bufs=1)
            nc.sync.dma_start(out=idx_sb[:], in_=idx.ap())

            call_i = 0
            for q in range(NQ):
                s8 = pool.tile([P, 8 * E], BF16, tag="s8")
                cq = NTILE * WA + q * WQ
                nc.gpsimd.dma_gather(
                    out_ap=s8[:].rearrange("p (g e) -> p g e", e=E),
                    in_ap=xs.ap(),
                    idxs_ap=idx_sb[:, cq : cq + WQ],
                    num_idxs=8 * P,
                    num_idxs_reg=8 * P,
                    elem_size=E,
                    queue_num=call_i % NQUEUES,
                )
                call_i += 1
                for ti in range(8):
                    t = q * 8 + ti
                    g = pool.tile([P, 8 * E], BF16, tag="g", bufs=3)
                    c0 = t * WA
                    nc.gpsimd.dma_gather(
                        out_ap=g[:].rearrange("p (g e) -> p g e", e=E),
                        in_ap=xs.ap(),
                        idxs_ap=idx_sb[:, c0 : c0 + WA],
                        num_idxs=NMAX,
                        num_idxs_reg=NMAX,
                        elem_size=E,
                        queue_num=call_i % NQUEUES,
                    )
                    call_i += 1
                    t4 = pool.tile([P, 4 * E], BF16, tag="t4")
                    nc.vector.tensor_tensor(
                        out=t4[:], in0=g[:, : 4 * E], in1=g[:, 4 * E :],
                        op=mybir.AluOpType.max,
                    )
                    t2 = pool.tile([P, 2 * E], BF16, tag="t2")
                    nc.vector.tensor_tensor(
                        out=t2[:], in0=t4[:, : 2 * E], in1=t4[:, 2 * E :],
                        op=mybir.AluOpType.max,
                    )
                    acc = pool.tile([P, E], BF16, tag="acc")
                    nc.vector.tensor_tensor(
                        out=acc[:], in0=t2[:, :E], in1=t2[:, E:],
                        op=mybir.AluOpType.max,
                    )
                    nc.vector.tensor_tensor(
                        out=acc[:], in0=acc[:], in1=s8[:, ti * E : (ti + 1) * E],
                        op=mybir.AluOpType.max,
                    )
                    nc.sync.dma_start(
                        out=out.ap()[t * P : (t + 1) * P, :], in_=acc[:]
                    )

    nc.compile()
    return nc


def _get_program():
    if "nc" not in _CACHE:
        _CACHE["nc"] = _build_program()
    return _CACHE["nc"]


def _wrap16(lst: np.ndarray) -> np.ndarray:
    """(N,) int -> (128, N/16) int16: 16-partition wrap, replicated x8."""
    w = len(lst) // 16
    return np.tile(lst.reshape(w, 16).T, (8, 1)).astype(np.int16)


def _host_prepare(x: np.ndarray, nb: np.ndarray) -> list[dict]:
    # (LIN, B*C) bf16: one 2KB row per input location
    xm = np.ascontiguousarray(x.transpose(2, 0, 1).reshape(LIN, E)).astype(
        ml_dtypes.bfloat16
    )

    in_maps = []
    for core in range(NCORE):
        nbc = nb[:, core * LPC : (core + 1) * LPC]        # (K, LPC)
        uniq, inv = np.unique(nbc, return_inverse=True)
        inv = inv.reshape(K, LPC)
        xs = np.zeros((UMAX, E), dtype=ml_dtypes.bfloat16)
        xs[: len(uniq)] = xm[uniq]
        cols = []
        for t in range(NTILE):
            loc2d = inv[:, t * P : (t + 1) * P]           # (K, P) local idx
            # per-tile call: slots 0..7 -> list[s*128+p] = loc2d[s, p]
            cols.append(_wrap16(loc2d[:8].ravel()))
        for q in range(NQ):
            # per-half slot-8 call: list[g*128+p] = inv[8, (q*8+g)*P + p]
            cols.append(_wrap16(inv[8, q * 8 * P : (q + 1) * 8 * P]))
        idx_np = np.ascontiguousarray(np.concatenate(cols, axis=1))
        in_maps.append({"xs": xs, "idx": idx_np})
    return in_maps


def kernel(x: np.ndarray, neighbours: np.ndarray) -> np.ndarray:
    x = np.asarray(x)
    nb = np.asarray(neighbours).astype(np.int64)          # (K, LOUT)
    assert x.shape == (B, C, LIN) and x.dtype == np.float32
    assert nb.shape == (K, LOUT)

    in_maps = _host_prepare(x, nb)
    nc = _get_program()
    res = bass_utils.run_bass_kernel_spmd(nc, in_maps, core_ids=list(range(NCORE)))
    _CACHE["last_result"] = res

    # out per core: (LPC, B*C) bf16 -> full (B, C, LOUT) f32
    dev = np.concatenate(
        [np.asarray(res.results[c]["out"]) for c in range(NCORE)]
    )  # (LOUT, E) bf16
    return np.ascontiguousarray(
        dev.reshape(LOUT, B, C).transpose(1, 2, 0)
    ).astype(np.float32)
